# revision 7
# baseline (speedup 1.0000x reference)
"""AttnDecoderRNN Trainium2 kernel, v2.

B=128 data-parallel over 8 cores (BL=16/core). Per core, the 16 batches
split into two phase-shifted groups of 8 (A leads, B lags half a step) so
the ACT engine's big tanh(U+q) for one group overlaps the PE/DVE tail
(softmax/ctx/gates/LSTM) of the other.

Per group-step:
  head:  qT = 0.5*Wa @ 2h (PE) -> DVE per-(b,hc) pre-add U+q (4x mode)
         -> ACT merged tanh per b -> X fp8 -> scores = (16*va)^T X via
         fp8 DoubleRow matmuls, 4 batches packed per PSUM tile (col groups)
  softmax: DVE strided-partition gathers (no DMA), max/exp(scale=1/16)/sum,
         normalization folded into the ctx gather
  tail:  wT via PE transpose -> fp8; ctx = w~^T enc via fp8 DoubleRow;
         gates into ONE [128,512] PSUM tile at col bands {0,32,64,96};
         single ACT tanh with per-partition scale (0.5/1.0) for all 4 gates;
         LSTM cell as 3 scalar_tensor_tensor ops (h,c stored as 2h,2c;
         0.5 folded into Wa/Whh/Wp host-side); y per-step DMA.
"""

import numpy as np
import ml_dtypes
from contextlib import ExitStack

import concourse.bass as bass
import concourse.tile as tile
from concourse import bacc, mybir
from concourse.bass_types import DynSlice
from concourse.bass_utils import run_bass_kernel_spmd

F32 = mybir.dt.float32
BF16 = mybir.dt.bfloat16
F8 = mybir.dt.float8e4
AF = mybir.ActivationFunctionType
ALU = mybir.AluOpType
AX = mybir.AxisListType
PM = mybir.MatmulPerfMode

B, T, H, D = 128, 512, 512, 128
NCORES = 8
BL = B // NCORES   # 16
GB = BL // 2       # 8 per group
HC = H // 128      # 4
TC = T // 128      # 4
G4 = 4 * H         # 2048
VA_SCALE = 16.0
HOOK_SLOTS = (1, 2, 3, 5)  # tail_start, tail_mid, tail_acts, tail_fin


def build(out_len: int, unroll: bool = False, bench_steps=None) -> bass.Bass:
    nc = bacc.Bacc(None, target_bir_lowering=False)

    encT = nc.dram_tensor("encT", [BL, HC, 128, T], BF16, kind="ExternalInput")
    uaT = nc.dram_tensor("uaT", [HC, 128, H], BF16, kind="ExternalInput")
    waT = nc.dram_tensor("waT", [HC, 128, H], BF16, kind="ExternalInput")
    whhT = nc.dram_tensor("whhT", [HC, 128, G4], BF16, kind="ExternalInput")
    wc8 = nc.dram_tensor("wc8", [HC, 128, G4], F8, kind="ExternalInput")
    wpT = nc.dram_tensor("wpT", [HC, 128, D], BF16, kind="ExternalInput")
    va8 = nc.dram_tensor("va8", [128, HC], F8, kind="ExternalInput")
    enc8 = nc.dram_tensor("enc8", [BL, TC, 128, H], F8, kind="ExternalInput")
    gcw = nc.dram_tensor("gcw", [BL, G4], BF16, kind="ExternalInput")
    bpw = nc.dram_tensor("bpw", [128, 1], F32, kind="ExternalInput")
    id8w = nc.dram_tensor("id8w", [8, 8], F32, kind="ExternalInput")
    id4w = nc.dram_tensor("id4w", [4, 4], F32, kind="ExternalInput")
    i16b = nc.dram_tensor("i16b", [16, 16], BF16, kind="ExternalInput")
    gscw = nc.dram_tensor("gscw", [128, 1], F32, kind="ExternalInput")
    yTA = nc.dram_tensor("yTA", [out_len, 128, GB], F32, kind="ExternalOutput")
    yTB = nc.dram_tensor("yTB", [out_len, 128, GB], F32, kind="ExternalOutput")

    with tile.TileContext(nc) as tc, ExitStack() as ctx:
        singles = ctx.enter_context(tc.tile_pool(name="singles", bufs=1))
        U_sb = singles.tile([128, BL, HC, T], BF16)        # 64KB/part
        enc8_sb = singles.tile([128, BL, TC, H], F8)       # 32KB/part
        waT_sb = singles.tile([128, HC, H], BF16)
        whhT_sb = singles.tile([128, HC, G4], BF16)
        wc8_sb = singles.tile([128, HC, G4], F8)
        wpT_sb = singles.tile([128, HC, D], BF16)
        va8_sb = singles.tile([128, HC], F8)
        gc_sb = singles.tile([BL, G4], BF16)
        bp_sb = singles.tile([128, 1], F32)
        id8_sb = singles.tile([8, 8], F32)
        id4_sb = singles.tile([4, 4], F32)
        i16b_sb = singles.tile([16, 16], BF16)
        gsc_sb = singles.tile([128, 1], F32)
        hsT_sb = singles.tile([128, HC, BL], BF16)         # 2h, transposed
        zcol_sb = singles.tile([1, 128], BF16)
        cs2_t = [singles.tile([128, H], F32, name=f"cs2_{gg}")
                 for gg in range(2)]                         # 2c at rows 32-39
        tgr_t = [singles.tile([GB, H], F32, name=f"tgr{gg}") for gg in range(2)]
        qT_sb = singles.tile([128, HC, BL], F32)

        nc.gpsimd.dma_start(out=waT_sb[:], in_=waT.rearrange("k p t -> p k t"))
        nc.gpsimd.dma_start(out=whhT_sb[:], in_=whhT.rearrange("k p t -> p k t"))
        nc.gpsimd.dma_start(out=wc8_sb[:], in_=wc8.rearrange("k p t -> p k t"))
        nc.gpsimd.dma_start(out=wpT_sb[:], in_=wpT.rearrange("k p t -> p k t"))
        nc.gpsimd.dma_start(out=va8_sb[:], in_=va8[:])
        nc.gpsimd.dma_start(out=gc_sb[:], in_=gcw[:])
        nc.gpsimd.dma_start(out=bp_sb[:], in_=bpw[:])
        nc.gpsimd.dma_start(out=id8_sb[:], in_=id8w[:])
        nc.gpsimd.dma_start(out=id4_sb[:], in_=id4w[:])
        nc.gpsimd.dma_start(out=i16b_sb[:], in_=i16b[:])
        nc.gpsimd.dma_start(out=gsc_sb[:], in_=gscw[:])
        for b in range(BL):
            nc.gpsimd.dma_start(out=enc8_sb[:, b], in_=enc8[b].rearrange("k p t -> p k t"))
        nc.vector.memset(hsT_sb[:], 0)
        nc.vector.memset(zcol_sb[:], 0)
        nc.vector.memset(cs2_t[0][:], 0)
        nc.vector.memset(cs2_t[1][:], 0)
        nc.vector.memset(qT_sb[:], 0)

        # PSUM pools: big 4 + {qp,gates} 2 + small 2 = 8 banks
        ps_big = ctx.enter_context(tc.tile_pool(name="ps_big", bufs=4, space="PSUM"))
        ps_g = ctx.enter_context(tc.tile_pool(name="ps_g", bufs=2, space="PSUM"))
        ps_sm = ctx.enter_context(tc.tile_pool(name="ps_sm", bufs=2, space="PSUM"))

        # --- pre-loop: U[b] = Ua @ enc[b]^T ---
        with tc.tile_pool(name="preloop", bufs=2) as prepool:
            uaT_sb = prepool.tile([128, HC, H], BF16, tag="uaw")
            nc.gpsimd.dma_start(out=uaT_sb[:], in_=uaT.rearrange("k p t -> p k t"))
            for b in range(BL):
                est = prepool.tile([128, HC, T], BF16, tag="est")
                nc.gpsimd.dma_start(out=est[:], in_=encT[b].rearrange("k p t -> p k t"))
                for mc in range(HC):
                    pu = ps_big.tile([128, T], F32, tag="big")
                    for kc in range(HC):
                        nc.tensor.matmul(
                            pu[:], uaT_sb[:, kc, mc * 128:(mc + 1) * 128],
                            est[:, kc, :], start=(kc == 0), stop=(kc == HC - 1))
                    nc.vector.tensor_copy(U_sb[:, b, mc, :], pu[:])

        tc.strict_bb_all_engine_barrier()

        work = ctx.enter_context(tc.tile_pool(name="work", bufs=2))
        w1 = ctx.enter_context(tc.tile_pool(name="w1", bufs=1))
        xpre_p = ctx.enter_context(tc.tile_pool(name="xpre", bufs=3))
        x8_p = ctx.enter_context(tc.tile_pool(name="x8", bufs=3))

        # persistent per-group tiles (bufs=1 pools keyed by tag)
        esc_t = {g: w1.tile([128, T], F32, tag=f"esc{g}", name=f"esc{g}") for g in range(2)}
        esp_t = {(g, h): w1.tile([128, T], F32, tag=f"esp{g}{h}", name=f"esp{g}{h}")
                 for g in range(2) for h in range(2)}
        csp_t = {(g, h): w1.tile([128, H], F32, tag=f"csp{g}{h}", name=f"csp{g}{h}")
                 for g in range(2) for h in range(2)}
        mneg_t = {g: w1.tile([128, 1], F32, tag=f"mneg{g}", name=f"mneg{g}") for g in range(2)}
        rcp_t = {g: w1.tile([128, 1], F32, tag=f"rcp{g}", name=f"rcp{g}") for g in range(2)}
        w8_t = {g: w1.tile([128, TC, GB], F8, tag=f"w8{g}", name=f"w8{g}") for g in range(2)}
        ctx_t = {g: w1.tile([128, H], F32, tag=f"ctx{g}", name=f"ctx{g}") for g in range(2)}
        c8_t = {g: w1.tile([128, HC, GB], F8, tag=f"c8{g}", name=f"c8{g}") for g in range(2)}
        gt_t = {g: w1.tile([128, T], F32, tag=f"gt{g}", name=f"gt{g}") for g in range(2)}
        t1_t = {g: w1.tile([128, H], F32, tag=f"t1{g}", name=f"t1{g}") for g in range(2)}
        t2_t = {g: w1.tile([128, H], F32, tag=f"t2{g}", name=f"t2{g}") for g in range(2)}
        tc_t = {g: w1.tile([128, H], F32, tag=f"tc{g}", name=f"tc{g}") for g in range(2)}
        h2_t = {g: w1.tile([GB, H], F32, tag=f"h2{g}", name=f"h2{g}") for g in range(2)}

        def bsl(g):  # batch column slice of group g
            return slice(g * GB, (g + 1) * GB)

        nc.vector.memset(esc_t[0][:], 0)
        nc.vector.memset(esc_t[1][:], 0)

        def head_q(g):
            # qT for group g; pre-adds read the PSUM scalar directly
            qp = ps_g.tile([128, HC, GB], F32, tag="g", name=f"qp{g}")
            for mc in range(HC):
                for kc in range(HC):
                    nc.tensor.matmul(
                        qp[:, mc, :], waT_sb[:, kc, mc * 128:(mc + 1) * 128],
                        hsT_sb[:, kc, bsl(g)], start=(kc == 0), stop=(kc == HC - 1))
            return qp

        sc_live = {}

        def softmax_half(g, half):
            # full-tile max/exp on the spread PSUM layout (rows {0,32,64,96}),
            # then a strided DMA densifies the 4 rows into esc rows 4*half..
            sc_ps = sc_live[g]
            mneg = mneg_t[g]
            esp = esp_t[(g, half)]
            nc.vector.tensor_reduce(mneg[:], sc_ps[half][:], axis=AX.X,
                                    op=ALU.max, negate=True)
            nc.vector.tensor_scalar(mneg[:], mneg[:],
                                    1.0 / VA_SCALE, None, ALU.mult)
            nc.scalar.activation(out=esp[:], in_=sc_ps[half][:],
                                 func=AF.Exp, bias=mneg[:],
                                 scale=1.0 / VA_SCALE)
            nc.sync.dma_start(out=esc_t[g][4 * half:4 * half + 4, :],
                              in_=esp[0:128:32, :])

        def head_x(g, qp, hooks=()):
            # per b: DVE pre-add, ACT merged tanh -> fp8, PE scores.
            # hooks[j] emits the other group's tail work into this group's
            # tanh window; own half0 softmax at j==5, half1 next window.
            sc_ps = [ps_big.tile([128, T], F32, tag="big", name=f"sc{g}{h}")
                     for h in range(2)]
            sc_live[g] = sc_ps
            for h in range(2):
                nc.tensor.matmul(sc_ps[h][:], zcol_sb[:], gc_sb[0:1, 0:T],
                                 start=True, stop=False, skip_group_check=True,
                                 tile_position=(0, 0))
            hd = {}
            for k, v in dict(hooks).items():
                hd.setdefault(k, []).extend(v)
            hooks = hd
            for j in range(GB):
                b = g * GB + j
                xp = xpre_p.tile([128, HC, T], BF16, tag="xpre", name=f"xp{b}")
                for hc in range(HC):
                    nc.vector.tensor_scalar(
                        xp[:, hc, :], U_sb[:, b, hc, :],
                        qp[:, hc, b - g * GB:b - g * GB + 1], None, ALU.add)
                x8 = x8_p.tile([128, HC, T], F8, tag="x8", name=f"x8{b}")
                nc.scalar.activation(
                    out=x8.rearrange("p k t -> p (k t)"),
                    in_=xp.rearrange("p k t -> p (k t)"),
                    func=AF.Tanh, bias=0.0, scale=1.0)
                half, jb = j // 4, j % 4
                for hc in range(HC):
                    nc.tensor.matmul(
                        sc_ps[half][32 * jb:32 * jb + 1, :],
                        va8_sb[:, hc:hc + 1], x8[:, hc, :],
                        start=False, stop=(hc == HC - 1),
                        skip_group_check=True, tile_position=(0, 32 * jb))
                if j == 5:
                    softmax_half(g, 0)
                if j == GB - 1:
                    softmax_half(g, 1)
                for fn in hooks.get(j, ()):
                    fn()

        def softmax_sum(g):
            nc.vector.tensor_reduce(rcp_t[g][0:GB, :], esc_t[g][0:GB, :],
                                    axis=AX.X, op=ALU.add)
            nc.vector.reciprocal(rcp_t[g][0:GB, :], rcp_t[g][0:GB, :])

        def tail_mms(g):
            # wT -> fp8; ctx fp8 DoubleRow; gather+normalize; ctxT -> fp8;
            # gates into one PSUM tile (bands).
            esc = esc_t[g]
            wtp = ps_sm.tile([128, TC, GB], F32, tag="sm")
            for t_c in range(TC):
                nc.tensor.transpose(wtp[:, t_c, :],
                                    esc[0:GB, t_c * 128:(t_c + 1) * 128],
                                    id8_sb[:])
            nc.vector.tensor_copy(
                w8_t[g].rearrange("p k b -> p (k b)"),
                wtp.rearrange("p k b -> p (k b)"))
            cx_ps = [ps_big.tile([128, H], F32, tag="big", name=f"cx{g}{h}")
                     for h in range(2)]
            for h in range(2):
                nc.tensor.matmul(cx_ps[h][:], zcol_sb[:], gc_sb[0:1, 0:H],
                                 start=True, stop=False, skip_group_check=True,
                                 tile_position=(0, 0))
            for j in range(GB):
                b = g * GB + j
                half, jb = j // 4, j % 4
                for t_c in range(TC):
                    nc.tensor.matmul(
                        cx_ps[half][32 * jb:32 * jb + 1, :],
                        w8_t[g][:, t_c, j:j + 1], enc8_sb[:, b, t_c, :],
                        start=False, stop=(t_c == TC - 1),
                        skip_group_check=True, tile_position=(0, 32 * jb))
            ctxs = ctx_t[g]
            for half in range(2):
                csp = csp_t[(g, half)]
                nc.vector.tensor_copy(csp[:], cx_ps[half][:])
                nc.sync.dma_start(out=ctxs[4 * half:4 * half + 4, :],
                                  in_=csp[0:128:32, :])
            nc.vector.tensor_scalar(ctxs[0:GB, :], ctxs[0:GB, :],
                                    rcp_t[g][0:GB, :], None, ALU.mult)
            ctp = ps_sm.tile([128, HC, GB], F32, tag="sm")
            for hc in range(HC):
                nc.tensor.transpose(ctp[:, hc, :],
                                    ctxs[0:GB, hc * 128:(hc + 1) * 128],
                                    id8_sb[:])
            nc.vector.tensor_copy(
                c8_t[g].rearrange("p k b -> p (k b)"),
                ctp.rearrange("p k b -> p (k b)"))
            gp = ps_g.tile([128, T], F32, tag="g", name=f"gp{g}")
            # zero-fill the whole tile (K=1 x zero weights) so the single
            # full-tile ACT tanh reads no uninitialized PSUM
            nc.tensor.matmul(gp[:], zcol_sb[:], gc_sb[0:1, 0:T], start=True,
                             stop=False, skip_group_check=True,
                             tile_position=(0, 0))
            for gi in range(4):
                col = 32 * gi
                gs = slice(gi * H, (gi + 1) * H)
                nc.tensor.matmul(gp[col:col + GB, :], i16b_sb[:, bsl(g)],
                                 gc_sb[:, gs], start=False, stop=False,
                                 skip_group_check=True, tile_position=(0, col))
                for kc in range(HC):
                    nc.tensor.matmul(gp[col:col + GB, :], hsT_sb[:, kc, bsl(g)],
                                     whhT_sb[:, kc, gs], start=False,
                                     stop=False, skip_group_check=True,
                                     tile_position=(0, col))
                # ctx part last: only these wait on this step's ctx
                for kc in range(HC):
                    nc.tensor.matmul(gp[col:col + GB, :], c8_t[g][:, kc, :],
                                     wc8_sb[:, kc, gs], start=False,
                                     stop=(kc == HC - 1),
                                     skip_group_check=True,
                                     tile_position=(0, col))
            return gp

        def tail_act_gt(g, gp):
            nc.scalar.activation(out=gt_t[g][:], in_=gp[:], func=AF.Tanh,
                                 bias=0.0, scale=gsc_sb[:])

        def tail_cell(g):
            # bands: i@0, f@32, g@64, o@96. cs2 lives at rows 32-39.
            # Legal-base plan: remap tg to base 0 (DVE copy), t1@32, t2
            # computed at base 0 with OUT remapped to 32, c2' at 32.
            gt = gt_t[g]
            cs = cs2_t[g][32:32 + GB, :]
            nc.vector.tensor_copy(tgr_t[g][:], gt[64:64 + GB, :])
            nc.vector.scalar_tensor_tensor(
                t1_t[g][32:32 + GB, :], gt[32:32 + GB, :], 1.0, cs,
                ALU.add, ALU.mult)
            nc.vector.scalar_tensor_tensor(
                t2_t[g][32:32 + GB, :], gt[0:GB, :], 1.0, tgr_t[g][:],
                ALU.add, ALU.mult)
            nc.vector.scalar_tensor_tensor(
                cs, t1_t[g][32:32 + GB, :], 0.5, t2_t[g][32:32 + GB, :],
                ALU.mult, ALU.add)

        def tail_act_tc(g):
            # ACT remap 32 -> 96 so h2 pairs with the o band
            nc.scalar.activation(out=tc_t[g][96:96 + GB, :],
                                 in_=cs2_t[g][32:32 + GB, :],
                                 func=AF.Tanh, bias=0.0, scale=0.5)

        def tail_h2(g):
            gt = gt_t[g]
            nc.vector.scalar_tensor_tensor(h2_t[g][:], gt[96:96 + GB, :], 1.0,
                                           tc_t[g][96:96 + GB, :],
                                           ALU.add, ALU.mult)

        def tail_finish(g, ysl):
            # h transpose + store, y matmul + DMA
            htp = ps_sm.tile([128, HC, GB], F32, tag="sm")
            for hc in range(HC):
                nc.tensor.transpose(htp[:, hc, :],
                                    h2_t[g][:, hc * 128:(hc + 1) * 128],
                                    id8_sb[:])
            nc.vector.tensor_copy(hsT_sb[:, :, bsl(g)], htp[:])
            yp = ps_sm.tile([128, GB], F32, tag="sm")
            for kc in range(HC):
                nc.tensor.matmul(yp[:], wpT_sb[:, kc, :], hsT_sb[:, kc, bsl(g)],
                                 start=(kc == 0), stop=(kc == HC - 1))
            ys = work.tile([128, GB], F32, tag=f"y{g}")
            nc.vector.tensor_scalar(ys[:], yp[:], bp_sb[:], None, ALU.add)
            yt = yTA if g == 0 else yTB
            nc.sync.dma_start(out=yt[ysl], in_=ys[:])

        def emit_iteration(iv, first):
            ysl_prev = (slice(0, 1) if bench_steps else DynSlice(iv - 1, 1))
            ysl_cur = (slice(0, 1) if bench_steps else DynSlice(iv, 1))
            gpx = [None, None]

            def tail_start(g):
                def f():
                    softmax_sum(g)
                return f

            def tail_mid(g):
                def f():
                    gpx[g] = tail_mms(g)
                return f

            def tail_acts(g):
                def f():
                    tail_act_gt(g, gpx[g])
                    tail_cell(g)
                    tail_act_tc(g)
                    tail_h2(g)
                return f

            def tail_fin(g, ysl):
                def f():
                    tail_finish(g, ysl)
                return f

            # A window: B' (previous step) tail slotted into A's tanh stream
            js, jm, ja, jf = HOOK_SLOTS
            qpA = head_q(0)
            hooksA = {} if first else {js: [], jm: [], ja: [], jf: []}
            if not first:
                hooksA[js] = hooksA[js] + [tail_start(1)]
                hooksA[jm] = hooksA[jm] + [tail_mid(1)]
                hooksA[ja] = hooksA[ja] + [tail_acts(1)]
                hooksA[jf] = hooksA[jf] + [tail_fin(1, ysl_prev)]
            head_x(0, qpA, tuple(hooksA.items()))
            # B window: A (current step) tail slotted into B's tanh stream
            qpB = head_q(1)
            hooksB = {js: [], jm: [], ja: [], jf: []}
            hooksB[js] = hooksB[js] + [tail_start(0)]
            hooksB[jm] = hooksB[jm] + [tail_mid(0)]
            hooksB[ja] = hooksB[ja] + [tail_acts(0)]
            hooksB[jf] = hooksB[jf] + [tail_fin(0, ysl_cur)]
            head_x(1, qpB, tuple(hooksB.items()))

        n = bench_steps or out_len
        # iteration 0 peeled (no B' tail yet)
        emit_iteration(0, True)
        if unroll:
            for i in range(1, n):
                emit_iteration(i, False)
        else:
            with tc.For_i(1, n, 1, hint_engines=(mybir.EngineType.PE,)) as i:
                emit_iteration(i, False)
        # epilogue: B tail for last step
        softmax_sum(1)
        gpB = tail_mms(1)
        tail_act_gt(1, gpB)
        tail_cell(1)
        tail_act_tc(1)
        tail_h2(1)
        tail_finish(1, slice(0, 1) if bench_steps else slice(n - 1, n))

    nc.finalize()
    return nc


_CACHE = {}


def _get_nc(out_len):
    if out_len not in _CACHE:
        _CACHE[out_len] = build(out_len)
    return _CACHE[out_len]


def make_inputs(encoder_outputs, latent_h, Wa, Ua, Va, W_ih, W_hh, b_ih, b_hh,
                Wp, bp):
    bf = ml_dtypes.bfloat16
    f8 = ml_dtypes.float8_e4m3
    enc = np.asarray(encoder_outputs, np.float32)
    Wa = np.asarray(Wa, np.float32)
    Ua = np.asarray(Ua, np.float32)
    Va = np.asarray(Va, np.float32)
    W_ih = np.asarray(W_ih, np.float32)
    W_hh = np.asarray(W_hh, np.float32)
    latent = np.asarray(latent_h, np.float32)

    encT_a = np.ascontiguousarray(
        enc.transpose(0, 2, 1).reshape(B, HC, 128, T)).astype(bf)
    uaT_a = np.ascontiguousarray(Ua.T.reshape(HC, 128, H)).astype(bf)
    waT_a = np.ascontiguousarray((0.5 * Wa.T).reshape(HC, 128, H)).astype(bf)
    whhT_a = np.ascontiguousarray(
        (0.5 * np.asarray(W_hh, np.float32).T).reshape(HC, 128, G4)).astype(bf)
    WcT = np.ascontiguousarray(W_ih[:, H:].T)  # (H, 4H)
    wc8_a = np.ascontiguousarray(WcT.reshape(HC, 128, G4)).astype(f8)
    wpT_a = np.ascontiguousarray(
        (0.5 * np.asarray(Wp, np.float32).T).reshape(HC, 128, D)).astype(bf)
    va16 = VA_SCALE * Va[0]
    va8_a = np.ascontiguousarray(va16.reshape(HC, 128).T).astype(f8)
    enc8_a = np.ascontiguousarray(enc.reshape(B, TC, 128, H)).astype(f8)
    gc_a = (latent @ W_ih[:, :H].T + np.asarray(b_ih, np.float32)
            + np.asarray(b_hh, np.float32)).astype(bf)
    bp_a = np.asarray(bp, np.float32).reshape(128, 1)
    id8_a = np.eye(8, dtype=np.float32)
    id4_a = np.eye(4, dtype=np.float32)
    i16b_a = np.eye(16).astype(bf)
    gsc_a = np.full((128, 1), 0.5, np.float32)
    gsc_a[64:64 + GB] = 1.0
    return dict(encT=encT_a, uaT=uaT_a, waT=waT_a, whhT=whhT_a, wc8=wc8_a,
                wpT=wpT_a, va8=va8_a, enc8=enc8_a, gcw=gc_a, bpw=bp_a,
                id8w=id8_a, id4w=id4_a, i16b=i16b_a, gscw=gsc_a)


def kernel(encoder_outputs, latent_h, Wa, Ua, Va, W_ih, W_hh, b_ih, b_hh,
           Wp, bp, out_len):
    out_len = int(out_len)
    full = make_inputs(encoder_outputs, latent_h, Wa, Ua, Va, W_ih, W_hh,
                       b_ih, b_hh, Wp, bp)
    shard_keys = ("encT", "enc8", "gcw")
    nc = _get_nc(out_len)
    in_maps = []
    for c in range(NCORES):
        s = slice(c * BL, (c + 1) * BL)
        m = {k: (v[s] if k in shard_keys else v) for k, v in full.items()}
        in_maps.append(m)
    import os
    trace = bool(os.environ.get("KERNEL_TRACE"))
    res = run_bass_kernel_spmd(nc, in_maps, core_ids=list(range(NCORES)),
                               trace=trace)
    if res.exec_time_ns is not None:
        print(f"HW exec time: {res.exec_time_ns} ns", flush=True)
    outs = []
    for r in res.results:
        ya = r["yTA"].transpose(2, 0, 1)  # (GB, out_len, D)
        yb = r["yTB"].transpose(2, 0, 1)
        outs.append(np.concatenate([ya, yb], axis=0))
    return np.concatenate(outs, axis=0).astype(np.float32)
